# revision 1
# baseline (speedup 1.0000x reference)
"""GQA attention kernel for 8 trn2 cores.

Sharding: core c -> (batch c//2, head-half c%2). Each core computes a partial
out-projection for its 8 KV heads / 4 query groups on one batch; host sums the
two half partials per batch and adds bo.

Device-side layout (per core):
  x^T   [128, 9, 2048]  bf16  e-major chunks; chunk 8 = ones row (bias trick)
  Q^T   [128, 4, 2048]  bf16  group g duplicated on both 64-row halves
  K^T   [128, 4, 2048]  bf16  pgroup g = heads (2g, 2g+1) on row halves
  Vones [128, 16, 8, 65] bf16 V natural + ones column (row-sum trick)
  P^T tiles [128, 16, 512] bf16 = exp(S^T) per (head, q-tile)
  attnout^T [128, 4, 2048] bf16 normalized
Scores are computed as S^T = K @ Q^T (contraction d=64) with two heads row-
packed in the PE array; PV contracts over k (partition dim) so no transposes
are needed anywhere. Softmax uses exp without max subtraction (scores are
O(1) for this problem) and the row-sum rides in the ones column of V.
"""

import numpy as np
import ml_dtypes

import concourse.bass as bass
import concourse.tile as tile
from concourse import bacc, mybir
from concourse.bass_utils import run_bass_kernel_spmd

B, S, E = 4, 2048, 1024
NH, NG, HD = 16, 8, 64
HPG = NH // NG            # heads per group = 2
SCALE = HD ** -0.5
NCORES = 8
HH = 8                    # heads per core
HG = 4                    # q-groups per core
EC = 9                    # e-chunks incl. bias/ones chunk
QT = 4                    # 512-wide q tiles
SB = 16                   # 128-row s blocks
KB = 16                   # 128-row k blocks

BF = mybir.dt.bfloat16
F32 = mybir.dt.float32

_CACHE = {}
LAST_RESULT = None


def _build_program():
    from contextlib import ExitStack

    nc = bacc.Bacc("TRN2", target_bir_lowering=False, debug=False)
    x_d = nc.dram_tensor("x", [S, E], BF, kind="ExternalInput").ap()
    wq_d = nc.dram_tensor("wq", [EC * 128, 512], BF, kind="ExternalInput").ap()
    wk_d = nc.dram_tensor("wk", [EC * 128, 512], BF, kind="ExternalInput").ap()
    wv_d = nc.dram_tensor("wv", [EC * 128, 512], BF, kind="ExternalInput").ap()
    wo_d = nc.dram_tensor("wo", [512, E], BF, kind="ExternalInput").ap()
    out_d = nc.dram_tensor("out", [S, E], F32, kind="ExternalOutput").ap()

    Exp = mybir.ActivationFunctionType.Exp

    with tile.TileContext(nc) as tc, ExitStack() as ctx:
        persist = ctx.enter_context(tc.tile_pool(name="persist", bufs=1))
        pt_pool = ctx.enter_context(tc.tile_pool(name="pt", bufs=3))
        small = ctx.enter_context(tc.tile_pool(name="small", bufs=2))
        outp = ctx.enter_context(tc.tile_pool(name="outp", bufs=2))
        ps512 = ctx.enter_context(tc.tile_pool(name="ps512", bufs=4, space="PSUM"))
        ps1k = ctx.enter_context(tc.tile_pool(name="ps1k", bufs=2, space="PSUM"))
        p1 = tc.tile_pool(name="p1", bufs=1)
        p1pool = p1.__enter__()

        # ---- phase-1-only SBUF tensors (freed before attention) ----
        xT = p1pool.tile([128, EC, S], BF, tag="xT")
        wq = p1pool.tile([128, EC, 512], BF, tag="wq")
        wk = p1pool.tile([128, EC, 512], BF, tag="wk")
        wv = p1pool.tile([128, EC, 512], BF, tag="wv")

        # ---- persistent SBUF tensors ----
        wo = persist.tile([128, 4, E], BF, tag="wo")
        QTr = persist.tile([128, HG, S], BF, tag="QTr")
        KT = persist.tile([128, HG, S], BF, tag="KT")
        Vones = persist.tile([128, SB, HH, HD + 1], BF, tag="Vones")
        aoT = persist.tile([128, 4, S], BF, tag="aoT")

        # ---- loads ----
        nc.sync.dma_start_transpose(xT[:, 0:8, :], x_d)
        nc.vector.memset(xT[:, 8, :], 0.0)
        nc.vector.memset(xT[0:1, 8, :], 1.0)
        nc.sync.dma_start(out=wq, in_=wq_d.rearrange("(c p) n -> p c n", p=128))
        nc.sync.dma_start(out=wk, in_=wk_d.rearrange("(c p) n -> p c n", p=128))
        nc.sync.dma_start(out=wv, in_=wv_d.rearrange("(c p) n -> p c n", p=128))
        nc.sync.dma_start(out=wo, in_=wo_d.rearrange("(c p) n -> p c n", p=128))
        nc.vector.memset(Vones[:, :, :, HD:HD + 1], 1.0)

        # ---- phase 1: projections ----
        for g in range(HG):
            for qt in range(QT):
                qs = slice(qt * 512, (qt + 1) * 512)
                ps = ps512.tile([128, 512], F32, tag="ps512")
                for c in range(EC):
                    nc.tensor.matmul(
                        ps, lhsT=wq[:, c, g * 128:(g + 1) * 128],
                        rhs=xT[:, c, qs], start=(c == 0), stop=(c == EC - 1))
                nc.vector.tensor_copy(out=QTr[:, g, qs], in_=ps)
                ps2 = ps512.tile([128, 512], F32, tag="ps512")
                for c in range(EC):
                    nc.tensor.matmul(
                        ps2, lhsT=wk[:, c, g * 128:(g + 1) * 128],
                        rhs=xT[:, c, qs], start=(c == 0), stop=(c == EC - 1))
                nc.vector.tensor_copy(out=KT[:, g, qs], in_=ps2)
        for sb in range(SB):
            ps = ps512.tile([128, 512], F32, tag="ps512")
            for c in range(EC):
                nc.tensor.matmul(
                    ps, lhsT=xT[:, c, sb * 128:(sb + 1) * 128],
                    rhs=wv[:, c, :], start=(c == 0), stop=(c == EC - 1))
            nc.vector.tensor_copy(
                out=Vones[:, sb, :, 0:HD],
                in_=ps.rearrange("p (h d) -> p h d", h=HH))
        p1.__exit__(None, None, None)

        # ---- phase 2: attention ----
        for g in range(HG):
            for qt in range(QT):
                qs = slice(qt * 512, (qt + 1) * 512)
                ptA = pt_pool.tile([128, KB, 512], BF, tag="PT")
                ptB = pt_pool.tile([128, KB, 512], BF, tag="PT")
                for kb2 in range(KB // 2):
                    sA = ps1k.tile([128, 1024], F32, tag="sc")
                    sB = ps1k.tile([128, 1024], F32, tag="sc")
                    for j in range(2):
                        kb = kb2 * 2 + j
                        ks = slice(kb * 128, (kb + 1) * 128)
                        js = slice(j * 512, (j + 1) * 512)
                        nc.tensor.matmul(
                            sA[:, js], lhsT=KT[0:64, g, ks],
                            rhs=QTr[0:64, g, qs], start=True, stop=True,
                            tile_position=(0, 0))
                        nc.tensor.matmul(
                            sB[:, js], lhsT=KT[64:128, g, ks],
                            rhs=QTr[64:128, g, qs], start=True, stop=True,
                            tile_position=(64, 0))
                    nc.scalar.activation(
                        out=ptA[:, kb2 * 2:kb2 * 2 + 2, :],
                        in_=sA.rearrange("p (k q) -> p k q", k=2), func=Exp)
                    nc.scalar.activation(
                        out=ptB[:, kb2 * 2:kb2 * 2 + 2, :],
                        in_=sB.rearrange("p (k q) -> p k q", k=2), func=Exp)
                pvA = ps512.tile([128, 512], F32, tag="ps512")
                pvB = ps512.tile([128, 512], F32, tag="ps512")
                for kb in range(KB):
                    nc.tensor.matmul(
                        pvA[0:HD + 1, :], lhsT=Vones[:, kb, 2 * g, :],
                        rhs=ptA[:, kb, :], start=(kb == 0), stop=(kb == KB - 1))
                    nc.tensor.matmul(
                        pvB[0:HD + 1, :], lhsT=Vones[:, kb, 2 * g + 1, :],
                        rhs=ptB[:, kb, :], start=(kb == 0), stop=(kb == KB - 1))
                for half, pv in ((0, pvA), (1, pvB)):
                    rr = small.tile([1, 512], F32, tag="recip")
                    nc.vector.reciprocal(out=rr, in_=pv[HD:HD + 1, :])
                    rep = small.tile([64, 512], F32, tag="rep")
                    nc.gpsimd.partition_broadcast(out_ap=rep, in_ap=rr)
                    nc.vector.tensor_mul(
                        out=aoT[half * 64:(half + 1) * 64, g, qs],
                        in0=pv[0:HD, :], in1=rep)

        # ---- phase 3: out-projection ----
        for sb in range(SB):
            ss = slice(sb * 128, (sb + 1) * 128)
            ot = outp.tile([128, E], F32, tag="ot")
            for et in range(2):
                es = slice(et * 512, (et + 1) * 512)
                ps = ps512.tile([128, 512], F32, tag="ps512")
                for c in range(4):
                    nc.tensor.matmul(
                        ps, lhsT=aoT[:, c, ss], rhs=wo[:, c, es],
                        start=(c == 0), stop=(c == 3))
                nc.vector.tensor_copy(out=ot[:, es], in_=ps)
            nc.sync.dma_start(out=out_d[ss, :], in_=ot)

    nc.compile()
    return nc


def _prep_shards(x, Wq, bq, Wk, bk, Wv, bv, Wo):
    """Host-side shard prep. Returns per-core input maps (bf16)."""
    bf16 = ml_dtypes.bfloat16
    xs = [np.ascontiguousarray(x[b]).astype(bf16) for b in range(B)]
    halves = []
    for half in range(2):
        # Wq: scale folded in, columns duplicated per group, bias row appended
        wq_cols = (Wq[:, half * 256:(half + 1) * 256] * SCALE).reshape(E, HG, HD)
        bq_h = (bq[half * 256:(half + 1) * 256] * SCALE).reshape(HG, HD)
        wq_f = np.zeros((EC * 128, 512), np.float32)
        wq_f[:E] = np.concatenate([wq_cols, wq_cols], axis=2).reshape(E, 512)
        wq_f[E] = np.concatenate([bq_h, bq_h], axis=1).reshape(512)

        wk_f = np.zeros((EC * 128, 512), np.float32)
        wk_f[:E] = Wk[:, half * 512:(half + 1) * 512]
        wk_f[E] = bk[half * 512:(half + 1) * 512]

        wv_f = np.zeros((EC * 128, 512), np.float32)
        wv_f[:E] = Wv[:, half * 512:(half + 1) * 512]
        wv_f[E] = bv[half * 512:(half + 1) * 512]

        wo_f = Wo[half * 512:(half + 1) * 512, :]
        halves.append({
            "wq": wq_f.astype(bf16), "wk": wk_f.astype(bf16),
            "wv": wv_f.astype(bf16), "wo": np.ascontiguousarray(wo_f).astype(bf16),
        })
    in_maps = []
    for c in range(NCORES):
        m = {"x": xs[c // 2]}
        m.update(halves[c % 2])
        in_maps.append(m)
    return in_maps


def kernel(x, Wq, bq, Wk, bk, Wv, bv, Wo, bo):
    global LAST_RESULT
    x, Wq, bq, Wk, bk, Wv, bv, Wo, bo = [
        np.asarray(a, dtype=np.float32)
        for a in (x, Wq, bq, Wk, bk, Wv, bv, Wo, bo)]
    if "nc" not in _CACHE:
        _CACHE["nc"] = _build_program()
    nc = _CACHE["nc"]
    in_maps = _prep_shards(x, Wq, bq, Wk, bk, Wv, bv, Wo)
    res = run_bass_kernel_spmd(nc, in_maps, core_ids=list(range(NCORES)))
    LAST_RESULT = res
    out = np.empty((B, S, E), np.float32)
    for b in range(B):
        out[b] = res.results[2 * b]["out"] + res.results[2 * b + 1]["out"]
    out += bo.astype(np.float32)
    return out

